# revision 4
# baseline (speedup 1.0000x reference)
"""DeepSeek MoE gate (sigmoid routing, grouped top-k) for 8x Trainium2 NeuronCores.

Strategy: data-parallel over tokens (16384 tokens -> 2048 per core), gate
weight + bias replicated. Numerics: fp32r (11-bit mantissa) main matmul at
bf16 speed + two fp8e4m3 DoubleRow correction matmuls at half cost:

  logits = xr @ wr  +  2^-18 * [ (2^12 xl)8 @ (2^6 w)8  +  x8 @ (2^18 wl)8 ]

where xr = rne11(x), xl = x - xr (exact in fp32), wr = rne11(w),
wl = w - wr. Per-term error ~2^-15 -> logit err ~6e-5, weight rel err
~1e-5 (validated vs fp64 on HW). Per x tile [128, 7168]:
  - stream 8-chunk pieces fp32 from HBM,
  - PE-transpose (fp32) into 4-chunk PSUM staging,
  - ACT rounds PSUM -> xr (f32r), DVE subtracts residual, GPSIMD scales
    residual into xl8 (e4m3), DVE casts PSUM -> x8 (e4m3),
  - matmuls per chunk-pair: 2x fp32r N=256 (main) + 2x fp8 DoubleRow
    (corrections) accumulating in separate PSUM banks,
  - combine (ACT scale + DVE add), sigmoid on ACT; +bias, grouped max,
    native top-8 (InstMax/InstMaxIndex), normalize on DVE.
"""

import os
import sys

sys.path.insert(0, "/opt/trn_rl_repo")

import numpy as np

import concourse.bass as bass
import concourse.mybir as mybir
import concourse.tile as tile
from concourse.bass_utils import run_bass_kernel_spmd
from concourse.masks import make_identity

P = 128
H = 7168
E = 256
G = 8  # n_group
GSZ = E // G  # 32 experts per group
TOPK_G = 4
TOPK = 8
N_CORES = 8
T_FULL = 4 * 4096
T_CORE = T_FULL // N_CORES
HC = H // P  # 56 contraction chunks
NPAIR = HC // 2  # 28 chunk pairs
NU = HC // 4  # 14 staging units of 4 chunks
PIECE = 8  # chunks per x DMA piece
NPIECE = HC // PIECE  # 7

F32 = mybir.dt.float32
F32R = mybir.dt.float32r
BF16 = mybir.dt.bfloat16
FP8 = mybir.dt.float8e4
U32 = mybir.dt.uint32

S_XL = 4096.0       # 2^12
S_W8 = 64.0         # 2^6
S_WL = 262144.0     # 2^18
S_INV = 1.0 / 262144.0


def build_moe_gate(tc: tile.TileContext, x_d, w_d, b_d, wout_d, iout_d, t_core,
                   ctx=None):
    nc = tc.nc
    nt = t_core // P

    const_pool = ctx.enter_context(tc.tile_pool(name="const", bufs=1))
    xin_pool = ctx.enter_context(tc.tile_pool(name="xin", bufs=8))
    win_pool = ctx.enter_context(tc.tile_pool(name="win", bufs=3))
    xr_pool = ctx.enter_context(tc.tile_pool(name="xr", bufs=32))
    xl8_pool = ctx.enter_context(tc.tile_pool(name="xl8", bufs=32))
    x8_pool = ctx.enter_context(tc.tile_pool(name="x8", bufs=32))
    scr_pool = ctx.enter_context(tc.tile_pool(name="scr", bufs=4))
    ps_t_pool = ctx.enter_context(tc.tile_pool(name="ps_t", bufs=3, space="PSUM"))
    ps_m_pool = ctx.enter_context(tc.tile_pool(name="ps_m", bufs=2, space="PSUM"))
    ps_c_pool = ctx.enter_context(tc.tile_pool(name="ps_c", bufs=2, space="PSUM"))
    sc_pool = ctx.enter_context(tc.tile_pool(name="scores", bufs=2))
    sm_pool = ctx.enter_context(tc.tile_pool(name="small", bufs=4))
    out_pool = ctx.enter_context(tc.tile_pool(name="outs", bufs=1))

    identity = const_pool.tile([P, P], F32)
    make_identity(nc, identity)

    # bias replicated across partitions: [128, 256]
    bias_rep = const_pool.tile([P, E], F32)
    nc.sync.dma_start(bias_rep, b_d[None, :].to_broadcast([P, E]))

    # W splits in [h, e] layout (one-time)
    wr_sb = const_pool.tile([P, HC, E], F32R)
    w8_sb = const_pool.tile([P, HC, E], FP8)
    wl8_sb = const_pool.tile([P, HC, E], FP8)

    def load_x_piece(i, pc):
        x_sb = xin_pool.tile([P, PIECE * P], F32, tag="xin")
        nc.sync.dma_start(
            x_sb, x_d[i * P:(i + 1) * P, pc * PIECE * P:(pc + 1) * PIECE * P])
        return x_sb

    def emit_x_unit(pieces, u, xr_pairs, xl8_pairs, x8_pairs):
        """Staging unit u (4 chunks = 2 pairs) for one x tile: 4 PE
        transposes into one PSUM bank, then per pair: ACT f32r-round,
        DVE residual, GPSIMD scaled fp8 residual, DVE fp8 cast."""
        src = pieces[u // 2]
        base = (u % 2) * 4  # chunk offset within the piece
        pt = ps_t_pool.tile([P, 4, P], F32, tag="ps_t")
        for q in range(4):
            nc.tensor.matmul(pt[:, q, :], src[:, (base + q) * P:(base + q + 1) * P],
                             identity, is_transpose=True,
                             start=(q == 0), stop=(q == 3))
        for h in range(2):
            pp = 2 * u + h  # global pair index in tile
            xr = xr_pool.tile([P, 2, P], F32R, tag="xr")
            nc.scalar.copy(xr, pt[:, 2 * h:2 * h + 2, :])
            xl = scr_pool.tile([P, 2, P], F32, tag="xl")
            nc.vector.tensor_sub(xl, pt[:, 2 * h:2 * h + 2, :], xr.bitcast(F32))
            xl8 = xl8_pool.tile([P, 2, P], FP8, tag="xl8")
            nc.gpsimd.tensor_scalar_mul(xl8, xl, S_XL)
            x8 = x8_pool.tile([P, 2, P], FP8, tag="x8")
            nc.vector.tensor_copy(x8, pt[:, 2 * h:2 * h + 2, :])
            xr_pairs[pp] = xr
            xl8_pairs[pp] = xl8
            x8_pairs[pp] = x8

    # ---- W prep (one-time): transpose W blocks, round/split ----
    # W DMAs ride the ACT HWDGE ring to stay off the x ring.
    def emit_w_unit(w_piece, eb, u_in_piece, jbase):
        """4 chunks of one W e-block: transpose + round/split into
        wr/w8/wl8 at chunk columns jbase..jbase+3."""
        pt = ps_t_pool.tile([P, 4, P], F32, tag="ps_t")
        for q in range(4):
            nc.tensor.matmul(pt[:, q, :],
                             w_piece[:, (u_in_piece * 4 + q) * P:
                                     (u_in_piece * 4 + q + 1) * P],
                             identity, is_transpose=True,
                             start=(q == 0), stop=(q == 3))
        for h in range(2):
            j0 = jbase + 2 * h
            dst = slice(eb * P, (eb + 1) * P)
            nc.scalar.copy(wr_sb[:, j0:j0 + 2, dst], pt[:, 2 * h:2 * h + 2, :])
            wl = scr_pool.tile([P, 2, P], F32, tag="wl")
            nc.vector.tensor_sub(wl, pt[:, 2 * h:2 * h + 2, :],
                                 wr_sb.bitcast(F32)[:, j0:j0 + 2, dst])
            nc.gpsimd.tensor_scalar_mul(wl8_sb[:, j0:j0 + 2, dst], wl, S_WL)
            nc.scalar.mul(w8_sb[:, j0:j0 + 2, dst], pt[:, 2 * h:2 * h + 2, :], S_W8)

    # ---- prologue: x tile 0 staging interleaved with W prep ----
    x0_pieces = [load_x_piece(0, pc) for pc in range(NPIECE)]
    w_pieces = {}
    for eb in range(2):
        for pc in range(NPIECE):
            wp = win_pool.tile([P, PIECE * P], F32, tag="win")
            nc.scalar.dma_start(
                wp, w_d[eb * P:(eb + 1) * P, pc * PIECE * P:(pc + 1) * PIECE * P])
            w_pieces[(eb, pc)] = wp

    xr_cur = {}
    xl8_cur = {}
    x8_cur = {}
    for u in range(NU):
        emit_x_unit(x0_pieces, u, xr_cur, xl8_cur, x8_cur)
        # interleave 2 W units per x unit (28 W units total)
        for k in range(2):
            wu = 2 * u + k
            eb, rem = divmod(wu, NU)
            emit_w_unit(w_pieces[(eb, rem // 2)], eb, rem % 2, (rem // 2) * 8 + (rem % 2) * 4)

    wout_sb = out_pool.tile([P, nt, TOPK], F32)
    iout_sb = out_pool.tile([P, nt, TOPK], U32)

    # ---- main loop over token tiles (software-pipelined) ----
    for i in range(nt):
        xr_p, xl8_p, x8_p = xr_cur, xl8_cur, x8_cur
        if i + 1 < nt:
            nxt_pieces = [load_x_piece(i + 1, pc) for pc in range(NPIECE)]
            xr_cur, xl8_cur, x8_cur = {}, {}, {}

        lg_m = ps_m_pool.tile([P, E], F32, tag="ps_m")
        lg_c = ps_c_pool.tile([P, E], F32, tag="ps_c")
        for u in range(NU):
            if i + 1 < nt:
                emit_x_unit(nxt_pieces, u, xr_cur, xl8_cur, x8_cur)
            for pp in (2 * u, 2 * u + 1):
                xr = xr_p[pp]
                for h in range(2):
                    j = 2 * pp + h
                    nc.tensor.matmul(lg_m, xr[:, h, :], wr_sb[:, j, :],
                                     start=(j == 0), stop=(j == HC - 1))
                jsl = slice(2 * pp, 2 * pp + 2)
                nc.tensor.matmul(lg_c, xl8_p[pp], w8_sb[:, jsl, :],
                                 start=(pp == 0), stop=False,
                                 perf_mode=mybir.MatmulPerfMode.DoubleRow)
                nc.tensor.matmul(lg_c, x8_p[pp], wl8_sb[:, jsl, :],
                                 start=False, stop=(pp == NPAIR - 1),
                                 perf_mode=mybir.MatmulPerfMode.DoubleRow)

        # logits = lg_m + lg_c * 2^-18; scores = sigmoid(logits) + bias
        corr_sc = sc_pool.tile([P, E], F32, tag="corr_sc")
        nc.scalar.mul(corr_sc, lg_c, S_INV)
        logits = sc_pool.tile([P, E], F32, tag="logits")
        nc.vector.tensor_add(logits, lg_m, corr_sc)
        scores = sc_pool.tile([P, E], F32, tag="scores")
        nc.scalar.activation(scores, logits, mybir.ActivationFunctionType.Sigmoid)
        nc.gpsimd.tensor_add(scores, scores, bias_rep)

        scores_g = scores.rearrange("p (g e) -> p g e", g=G)
        gmax = sm_pool.tile([P, G], F32, tag="gmax")
        nc.vector.reduce_max(gmax, scores_g, axis=mybir.AxisListType.X)

        g8 = sm_pool.tile([P, 8], F32, tag="g8")
        nc.vector.max(out=g8, in_=gmax)

        # group mask: 1.0 where group score >= 4th-largest group score
        gmask = sm_pool.tile([P, G], F32, tag="gmask")
        nc.vector.tensor_scalar(gmask, gmax, g8[:, TOPK_G - 1:TOPK_G], None,
                                op0=mybir.AluOpType.is_ge)

        masked = sc_pool.tile([P, E], F32, tag="masked")
        nc.gpsimd.tensor_tensor(
            masked.rearrange("p (g e) -> p g e", g=G), scores_g,
            gmask[:, :, None].to_broadcast([P, G, GSZ]),
            op=mybir.AluOpType.mult)

        m8 = sm_pool.tile([P, 8], F32, tag="m8")
        nc.vector.max(out=m8, in_=masked)
        nc.vector.max_index(iout_sb[:, i, :], m8, masked)

        ssum = sm_pool.tile([P, 1], F32, tag="ssum")
        nc.vector.reduce_sum(ssum, m8, axis=mybir.AxisListType.X)
        nc.vector.tensor_scalar_add(ssum, ssum, 1e-6)
        rcp = sm_pool.tile([P, 1], F32, tag="rcp")
        nc.vector.reciprocal(rcp, ssum)
        nc.vector.tensor_scalar_mul(wout_sb[:, i, :], m8, rcp)

    # outputs in [p, n, k] layout; host reorders to [n*p, k]
    nc.sync.dma_start(wout_d, wout_sb)
    nc.sync.dma_start(iout_d, iout_sb)


def build_bass(t_core=T_CORE):
    from concourse import bacc
    nc = bacc.Bacc("TRN2", target_bir_lowering=False, debug=False,
                   num_devices=N_CORES)
    nt = t_core // P
    x_d = nc.dram_tensor("x", [t_core, H], F32, kind="ExternalInput").ap()
    w_d = nc.dram_tensor("w", [E, H], F32, kind="ExternalInput").ap()
    b_d = nc.dram_tensor("b", [E], F32, kind="ExternalInput").ap()
    wout_d = nc.dram_tensor("wout", [P, nt, TOPK], F32,
                            kind="ExternalOutput").ap()
    iout_d = nc.dram_tensor("iout", [P, nt, TOPK], U32,
                            kind="ExternalOutput").ap()
    from contextlib import ExitStack
    with tile.TileContext(nc) as tc:
        with ExitStack() as ctx:
            build_moe_gate(tc, x_d, w_d, b_d, wout_d, iout_d, t_core, ctx=ctx)
    nc.compile()
    return nc


_NC_CACHE = {}


def _get_nc():
    key = "main"
    if key not in _NC_CACHE:
        _NC_CACHE[key] = build_bass()
    return _NC_CACHE[key]


def kernel(hidden_states, gate_weight, bias, n_group, topk_group, top_k,
           _trace=False):
    assert int(n_group) == G and int(topk_group) == TOPK_G and int(top_k) == TOPK
    x = np.asarray(hidden_states, dtype=np.float32)
    w = np.ascontiguousarray(np.asarray(gate_weight, dtype=np.float32))
    b = np.ascontiguousarray(np.asarray(bias, dtype=np.float32))
    B, S, _ = x.shape
    xf = x.reshape(-1, H)
    assert xf.shape[0] == T_FULL

    nc = _get_nc()
    in_maps = []
    for c in range(N_CORES):
        in_maps.append({
            "x": np.ascontiguousarray(xf[c * T_CORE:(c + 1) * T_CORE]),
            "w": w,
            "b": b,
        })
    try:
        res = run_bass_kernel_spmd(nc, in_maps, core_ids=list(range(N_CORES)),
                                   trace=_trace)
    except ModuleNotFoundError:
        # axon NTFF profiling hook unavailable in this container
        res = run_bass_kernel_spmd(nc, in_maps, core_ids=list(range(N_CORES)),
                                   trace=False)
    weights = np.empty((T_FULL, TOPK), dtype=np.float32)
    indices = np.empty((T_FULL, TOPK), dtype=np.int32)
    for c, r in enumerate(res.results):
        # [P, nt, K] -> [nt*P, K]
        wc = np.transpose(r["wout"], (1, 0, 2)).reshape(T_CORE, TOPK)
        ic = np.transpose(r["iout"], (1, 0, 2)).reshape(T_CORE, TOPK)
        weights[c * T_CORE:(c + 1) * T_CORE] = wc
        indices[c * T_CORE:(c + 1) * T_CORE] = ic.astype(np.int32)
    out_w = weights.reshape(B, S, TOPK)
    out_i = indices.reshape(B, S, TOPK)
    if _trace:
        return (out_w, out_i), res
    return out_w, out_i


# revision 7
# speedup vs baseline: 5.2638x; 5.2638x over previous
"""DeepSeek MoE gate (sigmoid routing, grouped top-k) for 8x Trainium2 NeuronCores.

Data-parallel over tokens (16384 -> 2048/core), gate weight + bias replicated.

Numerics: fp16 main matmul (11-bit mantissa, 1 cycle/row — same PE speed as
bf16) plus one fp8e5m2 DoubleRow correction pass (~0.5 cycle/row/chunk):

  logits = xh16 @ wh16 + 2^-6 * [ xl8 @ (2^6 w)8 + x8 @ (2^6 wl)8 ]

with xh16 = fp16(x), xl8 = e5m2(x - xh16) (exact residual, quantized),
wh16 = fp16(w), wl8 = e5m2(2^6 (w - wh16)), w8 = e5m2(2^6 w), x8 = e5m2(xh16).
Per-term error ~2^-15 -> logit err ~1e-4 class, far inside the 2e-2 gate.

Per x tile [128 tokens, 7168]:
  phase A: 8-chunk units: fp32 PE transpose -> PSUM; ACT casts PSUM->xh16,
           DVE subtracts residual straight to e5m2, ACT casts xh16->x8.
  phase B: 56 fp16 matmuls N=256 (uniform mode, no PE mode switching).
  phase C: 28 fp8e5 DoubleRow matmuls (2 chunks each) into a second PSUM bank.
  tail: combine + sigmoid + bias + grouped top-k + normalize.
Emission order per tile i: DMA(i+1), B(i), A(i+1), C(i) — PE never waits on
fresh DMA, modes stay blocked, ~3 mode switches/tile.
"""

import sys

sys.path.insert(0, "/opt/trn_rl_repo")

import numpy as np

import concourse.bass as bass
import concourse.mybir as mybir
import concourse.tile as tile
from concourse.bass_utils import run_bass_kernel_spmd
from concourse.masks import make_identity

P = 128
H = 7168
E = 256
G = 8
GSZ = E // G
TOPK_G = 4
TOPK = 8
N_CORES = 8
T_FULL = 4 * 4096
T_CORE = T_FULL // N_CORES
HC = H // P          # 56 contraction chunks
NPAIR = HC // 2      # 28 DoubleRow pairs
UCH = 8              # chunks per staging unit
NU = HC // UCH       # 7 units per tile
PIECE = 8            # chunks per x DMA piece
NPIECE = HC // PIECE

F32 = mybir.dt.float32
FP16 = mybir.dt.float16
FP8 = mybir.dt.float8e5
U32 = mybir.dt.uint32

S_W = 64.0           # 2^6 scale on the W-side fp8 operands
S_INV = 1.0 / 64.0


def build_moe_gate(tc: tile.TileContext, x_d, w_d, b_d, wout_d, iout_d, t_core,
                   ctx=None):
    nc = tc.nc
    nt = t_core // P

    const_pool = ctx.enter_context(tc.tile_pool(name="const", bufs=1))
    xin_pool = ctx.enter_context(tc.tile_pool(name="xin", bufs=8))
    win_pool = ctx.enter_context(tc.tile_pool(name="win", bufs=4))
    xh_pool = ctx.enter_context(tc.tile_pool(name="xh", bufs=2))
    scr_pool = ctx.enter_context(tc.tile_pool(name="scr", bufs=3))
    ps_t_pool = ctx.enter_context(tc.tile_pool(name="ps_t", bufs=2, space="PSUM"))
    ps_m_pool = ctx.enter_context(tc.tile_pool(name="ps_m", bufs=2, space="PSUM"))
    ps_c_pool = ctx.enter_context(tc.tile_pool(name="ps_c", bufs=2, space="PSUM"))
    sc_pool = ctx.enter_context(tc.tile_pool(name="scores", bufs=2))
    sm_pool = ctx.enter_context(tc.tile_pool(name="small", bufs=4))
    out_pool = ctx.enter_context(tc.tile_pool(name="outs", bufs=1))

    identity = const_pool.tile([P, P], F32)
    make_identity(nc, identity)

    bias_rep = const_pool.tile([P, E], F32)
    nc.sync.dma_start(bias_rep, b_d[None, :].to_broadcast([P, E]))

    # W splits in [h, e] layout (one-time)
    wh_sb = const_pool.tile([P, HC, E], FP16)
    w8_sb = const_pool.tile([P, HC, E], FP8)
    wl8_sb = const_pool.tile([P, HC, E], FP8)

    def load_x_piece(i, pc):
        x_sb = xin_pool.tile([P, PIECE * P], F32, tag="xin")
        nc.sync.dma_start(
            x_sb, x_d[i * P:(i + 1) * P, pc * PIECE * P:(pc + 1) * PIECE * P])
        return x_sb

    def emit_x_unit(pieces, u, xh, xl8, x8):
        """Unit u (8 chunks): fp32 transposes into one PSUM staging tile,
        then ACT round->fp16, DVE residual->e5m2, ACT xh16->e5m2."""
        src = pieces[u]
        pt = ps_t_pool.tile([P, UCH, P], F32, tag="ps_t")
        for q in range(UCH):
            nc.tensor.matmul(pt[:, q, :], src[:, q * P:(q + 1) * P],
                             identity, is_transpose=True,
                             start=(q % 4 == 0), stop=(q % 4 == 3))
        csl = slice(u * UCH, (u + 1) * UCH)
        nc.scalar.copy(xh[:, csl, :], pt)
        nc.vector.tensor_sub(xl8[:, csl, :], pt, xh[:, csl, :])
        nc.scalar.copy(x8[:, csl, :], xh[:, csl, :])

    def emit_w_unit(w_piece, eb, jbase):
        """8 chunks of one W e-block: transpose + split into wh/w8/wl8."""
        pt = ps_t_pool.tile([P, UCH, P], F32, tag="ps_t")
        for q in range(UCH):
            nc.tensor.matmul(pt[:, q, :], w_piece[:, q * P:(q + 1) * P],
                             identity, is_transpose=True,
                             start=(q % 4 == 0), stop=(q % 4 == 3))
        dst = slice(eb * P, (eb + 1) * P)
        jsl = slice(jbase, jbase + UCH)
        nc.scalar.copy(wh_sb[:, jsl, dst], pt)
        wl = scr_pool.tile([P, UCH, P], F32, tag="wl")
        nc.vector.tensor_sub(wl, pt, wh_sb[:, jsl, dst])
        nc.scalar.mul(wl8_sb[:, jsl, dst], wl, S_W)
        nc.scalar.mul(w8_sb[:, jsl, dst], pt, S_W)

    # ---- prologue: x tile 0 staging interleaved with W prep ----
    x0_pieces = [load_x_piece(0, pc) for pc in range(NPIECE)]
    w_pieces = {}
    for pc in range(NPIECE):
        for eb in range(2):
            wp = win_pool.tile([P, PIECE * P], F32, tag="win")
            nc.scalar.dma_start(
                wp, w_d[eb * P:(eb + 1) * P, pc * PIECE * P:(pc + 1) * PIECE * P])
            w_pieces[(eb, pc)] = wp

    xh_cur = xh_pool.tile([P, HC, P], FP16, tag="xh")
    xl8_cur = xh_pool.tile([P, HC, P], FP8, tag="xl8")
    x8_cur = xh_pool.tile([P, HC, P], FP8, tag="x8")
    for u in range(NU):
        emit_x_unit(x0_pieces, u, xh_cur, xl8_cur, x8_cur)
        emit_w_unit(w_pieces[(0, u)], 0, u * UCH)
        emit_w_unit(w_pieces[(1, u)], 1, u * UCH)

    wout_sb = out_pool.tile([P, nt, TOPK], F32)
    iout_sb = out_pool.tile([P, nt, TOPK], U32)

    # ---- main loop over token tiles ----
    for i in range(nt):
        xh_p, xl8_p, x8_p = xh_cur, xl8_cur, x8_cur
        if i + 1 < nt:
            nxt_pieces = [load_x_piece(i + 1, pc) for pc in range(NPIECE)]
            xh_cur = xh_pool.tile([P, HC, P], FP16, tag="xh")
            xl8_cur = xh_pool.tile([P, HC, P], FP8, tag="xl8")
            x8_cur = xh_pool.tile([P, HC, P], FP8, tag="x8")

        # phase B: fp16 main matmuls (uniform mode block)
        lg_m = ps_m_pool.tile([P, E], F32, tag="ps_m")
        for j in range(HC):
            nc.tensor.matmul(lg_m, xh_p[:, j, :], wh_sb[:, j, :],
                             start=(j == 0), stop=(j == HC - 1))

        # phase A for tile i+1 (fp32 transpose block)
        if i + 1 < nt:
            for u in range(NU):
                emit_x_unit(nxt_pieces, u, xh_cur, xl8_cur, x8_cur)

        # phase C: fp8e5 DoubleRow corrections (uniform mode block)
        lg_c = ps_c_pool.tile([P, E], F32, tag="ps_c")
        for pp in range(NPAIR):
            jsl = slice(2 * pp, 2 * pp + 2)
            nc.tensor.matmul(lg_c, xl8_p[:, jsl, :], w8_sb[:, jsl, :],
                             start=(pp == 0), stop=False,
                             perf_mode=mybir.MatmulPerfMode.DoubleRow)
            nc.tensor.matmul(lg_c, x8_p[:, jsl, :], wl8_sb[:, jsl, :],
                             start=False, stop=(pp == NPAIR - 1),
                             perf_mode=mybir.MatmulPerfMode.DoubleRow)

        # tail: logits = lg_m + lg_c/64; scores = sigmoid + bias
        corr_sc = sc_pool.tile([P, E], F32, tag="corr_sc")
        nc.scalar.mul(corr_sc, lg_c, S_INV)
        logits = sc_pool.tile([P, E], F32, tag="logits")
        nc.vector.tensor_add(logits, lg_m, corr_sc)
        scores = sc_pool.tile([P, E], F32, tag="scores")
        nc.scalar.activation(scores, logits, mybir.ActivationFunctionType.Sigmoid)
        nc.gpsimd.tensor_add(scores, scores, bias_rep)

        scores_g = scores.rearrange("p (g e) -> p g e", g=G)
        gmax = sm_pool.tile([P, G], F32, tag="gmax")
        nc.vector.reduce_max(gmax, scores_g, axis=mybir.AxisListType.X)

        g8 = sm_pool.tile([P, 8], F32, tag="g8")
        nc.vector.max(out=g8, in_=gmax)

        gmask = sm_pool.tile([P, G], F32, tag="gmask")
        nc.vector.tensor_scalar(gmask, gmax, g8[:, TOPK_G - 1:TOPK_G], None,
                                op0=mybir.AluOpType.is_ge)

        masked = sc_pool.tile([P, E], F32, tag="masked")
        nc.gpsimd.tensor_tensor(
            masked.rearrange("p (g e) -> p g e", g=G), scores_g,
            gmask[:, :, None].to_broadcast([P, G, GSZ]),
            op=mybir.AluOpType.mult)

        m8 = sm_pool.tile([P, 8], F32, tag="m8")
        nc.vector.max(out=m8, in_=masked)
        nc.vector.max_index(iout_sb[:, i, :], m8, masked)

        ssum = sm_pool.tile([P, 1], F32, tag="ssum")
        nc.vector.reduce_sum(ssum, m8, axis=mybir.AxisListType.X)
        nc.vector.tensor_scalar_add(ssum, ssum, 1e-6)
        rcp = sm_pool.tile([P, 1], F32, tag="rcp")
        nc.vector.reciprocal(rcp, ssum)
        nc.vector.tensor_scalar_mul(wout_sb[:, i, :], m8, rcp)

    nc.sync.dma_start(wout_d, wout_sb)
    nc.sync.dma_start(iout_d, iout_sb)


def build_bass(t_core=T_CORE):
    from concourse import bacc
    nc = bacc.Bacc("TRN2", target_bir_lowering=False, debug=False,
                   num_devices=N_CORES)
    nt = t_core // P
    x_d = nc.dram_tensor("x", [t_core, H], F32, kind="ExternalInput").ap()
    w_d = nc.dram_tensor("w", [E, H], F32, kind="ExternalInput").ap()
    b_d = nc.dram_tensor("b", [E], F32, kind="ExternalInput").ap()
    wout_d = nc.dram_tensor("wout", [P, nt, TOPK], F32,
                            kind="ExternalOutput").ap()
    iout_d = nc.dram_tensor("iout", [P, nt, TOPK], U32,
                            kind="ExternalOutput").ap()
    from contextlib import ExitStack
    with tile.TileContext(nc) as tc:
        with ExitStack() as ctx:
            build_moe_gate(tc, x_d, w_d, b_d, wout_d, iout_d, t_core, ctx=ctx)
    nc.compile()
    return nc


_NC_CACHE = {}


def _get_nc():
    key = "main"
    if key not in _NC_CACHE:
        _NC_CACHE[key] = build_bass()
    return _NC_CACHE[key]


def kernel(hidden_states, gate_weight, bias, n_group, topk_group, top_k,
           _trace=False):
    assert int(n_group) == G and int(topk_group) == TOPK_G and int(top_k) == TOPK
    x = np.asarray(hidden_states, dtype=np.float32)
    w = np.ascontiguousarray(np.asarray(gate_weight, dtype=np.float32))
    b = np.ascontiguousarray(np.asarray(bias, dtype=np.float32))
    B, S, _ = x.shape
    xf = x.reshape(-1, H)
    assert xf.shape[0] == T_FULL

    nc = _get_nc()
    in_maps = []
    for c in range(N_CORES):
        in_maps.append({
            "x": np.ascontiguousarray(xf[c * T_CORE:(c + 1) * T_CORE]),
            "w": w,
            "b": b,
        })
    try:
        res = run_bass_kernel_spmd(nc, in_maps, core_ids=list(range(N_CORES)),
                                   trace=_trace)
    except ModuleNotFoundError:
        res = run_bass_kernel_spmd(nc, in_maps, core_ids=list(range(N_CORES)),
                                   trace=False)
    weights = np.empty((T_FULL, TOPK), dtype=np.float32)
    indices = np.empty((T_FULL, TOPK), dtype=np.int32)
    for c, r in enumerate(res.results):
        wc = np.transpose(r["wout"], (1, 0, 2)).reshape(T_CORE, TOPK)
        ic = np.transpose(r["iout"], (1, 0, 2)).reshape(T_CORE, TOPK)
        weights[c * T_CORE:(c + 1) * T_CORE] = wc
        indices[c * T_CORE:(c + 1) * T_CORE] = ic.astype(np.int32)
    out_w = weights.reshape(B, S, TOPK)
    out_i = indices.reshape(B, S, TOPK)
    if _trace:
        return (out_w, out_i), res
    return out_w, out_i


# revision 8
# speedup vs baseline: 5.2696x; 1.0011x over previous
"""DeepSeek MoE gate (sigmoid routing, grouped top-k) for 8x Trainium2 NeuronCores.

Data-parallel over tokens (16384 -> 2048/core), gate weight + bias replicated.

Numerics: fp16 main matmul (11-bit mantissa, 1 cycle/row — same PE speed as
bf16) plus one fp8e5m2 DoubleRow correction pass (~0.5 cycle/row/chunk):

  logits = xh16 @ wh16 + 2^-6 * [ xl8 @ (2^6 w)8 + x8 @ (2^6 wl)8 ]

with xh16 = fp16(x), xl8 = e5m2(x - xh16) (exact residual, quantized),
wh16 = fp16(w), wl8 = e5m2(2^6 (w - wh16)), w8 = e5m2(2^6 w), x8 = e5m2(xh16).
Per-term error ~2^-15 -> logit err ~1e-4 class, far inside the 2e-2 gate.

Per x tile [128 tokens, 7168]:
  phase A: 8-chunk units: fp32 PE transpose -> PSUM; ACT casts PSUM->xh16,
           DVE subtracts residual straight to e5m2, ACT casts xh16->x8.
  phase B: 56 fp16 matmuls N=256 (uniform mode, no PE mode switching).
  phase C: 28 fp8e5 DoubleRow matmuls (2 chunks each) into a second PSUM bank.
  tail: combine + sigmoid + bias + grouped top-k + normalize.
Emission order per tile i: DMA(i+1), B(i), A(i+1), C(i) — PE never waits on
fresh DMA, modes stay blocked, ~3 mode switches/tile.
"""

import sys

sys.path.insert(0, "/opt/trn_rl_repo")

import numpy as np

import concourse.bass as bass
import concourse.mybir as mybir
import concourse.tile as tile
from concourse.bass_utils import run_bass_kernel_spmd
from concourse.masks import make_identity

P = 128
H = 7168
E = 256
G = 8
GSZ = E // G
TOPK_G = 4
TOPK = 8
N_CORES = 8
T_FULL = 4 * 4096
T_CORE = T_FULL // N_CORES
HC = H // P          # 56 contraction chunks
NPAIR = HC // 2      # 28 DoubleRow pairs
UCH = 8              # chunks per staging unit
NU = HC // UCH       # 7 units per tile
PIECE = 8            # chunks per x DMA piece
NPIECE = HC // PIECE

F32 = mybir.dt.float32
FP16 = mybir.dt.float16
FP8 = mybir.dt.float8e5   # residual operands (need denormal range)
FP8H = mybir.dt.float8e4  # high operands (one extra mantissa bit)
U32 = mybir.dt.uint32

S_W = 64.0           # 2^6 scale on the W-side fp8 operands
S_INV = 1.0 / 64.0


def build_moe_gate(tc: tile.TileContext, x_d, w_d, b_d, wout_d, iout_d, t_core,
                   ctx=None):
    nc = tc.nc
    nt = t_core // P

    const_pool = ctx.enter_context(tc.tile_pool(name="const", bufs=1))
    xin_pool = ctx.enter_context(tc.tile_pool(name="xin", bufs=8))
    win_pool = ctx.enter_context(tc.tile_pool(name="win", bufs=4))
    xh_pool = ctx.enter_context(tc.tile_pool(name="xh", bufs=2))
    scr_pool = ctx.enter_context(tc.tile_pool(name="scr", bufs=3))
    ps_t_pool = ctx.enter_context(tc.tile_pool(name="ps_t", bufs=2, space="PSUM"))
    ps_m_pool = ctx.enter_context(tc.tile_pool(name="ps_m", bufs=2, space="PSUM"))
    ps_c_pool = ctx.enter_context(tc.tile_pool(name="ps_c", bufs=2, space="PSUM"))
    sc_pool = ctx.enter_context(tc.tile_pool(name="scores", bufs=2))
    sm_pool = ctx.enter_context(tc.tile_pool(name="small", bufs=4))
    out_pool = ctx.enter_context(tc.tile_pool(name="outs", bufs=1))

    identity = const_pool.tile([P, P], F32)
    make_identity(nc, identity)

    bias_rep = const_pool.tile([P, E], F32)
    nc.sync.dma_start(bias_rep, b_d[None, :].to_broadcast([P, E]))

    # W splits in [h, e] layout (one-time)
    wh_sb = const_pool.tile([P, HC, E], FP16)
    w8_sb = const_pool.tile([P, HC, E], FP8H)
    wl8_sb = const_pool.tile([P, HC, E], FP8)

    def load_x_piece(i, pc):
        x_sb = xin_pool.tile([P, PIECE * P], F32, tag="xin")
        nc.sync.dma_start(
            x_sb, x_d[i * P:(i + 1) * P, pc * PIECE * P:(pc + 1) * PIECE * P])
        return x_sb

    def emit_x_unit(pieces, u, xh, xl8, x8):
        """Unit u (8 chunks): fp32 transposes into one PSUM staging tile,
        then ACT round->fp16, DVE residual->e5m2, ACT xh16->e5m2."""
        src = pieces[u]
        pt = ps_t_pool.tile([P, UCH, P], F32, tag="ps_t")
        for q in range(UCH):
            nc.tensor.matmul(pt[:, q, :], src[:, q * P:(q + 1) * P],
                             identity, is_transpose=True,
                             start=(q % 4 == 0), stop=(q % 4 == 3))
        csl = slice(u * UCH, (u + 1) * UCH)
        nc.scalar.copy(xh[:, csl, :], pt)
        nc.vector.tensor_sub(xl8[:, csl, :], pt, xh[:, csl, :])
        nc.scalar.copy(x8[:, csl, :], xh[:, csl, :])

    def emit_w_unit(w_piece, eb, jbase):
        """8 chunks of one W e-block: transpose + split into wh/w8/wl8."""
        pt = ps_t_pool.tile([P, UCH, P], F32, tag="ps_t")
        for q in range(UCH):
            nc.tensor.matmul(pt[:, q, :], w_piece[:, q * P:(q + 1) * P],
                             identity, is_transpose=True,
                             start=(q % 4 == 0), stop=(q % 4 == 3))
        dst = slice(eb * P, (eb + 1) * P)
        jsl = slice(jbase, jbase + UCH)
        nc.scalar.copy(wh_sb[:, jsl, dst], pt)
        wl = scr_pool.tile([P, UCH, P], F32, tag="wl")
        nc.vector.tensor_sub(wl, pt, wh_sb[:, jsl, dst])
        nc.scalar.mul(wl8_sb[:, jsl, dst], wl, S_W)
        nc.scalar.mul(w8_sb[:, jsl, dst], pt, S_W)

    # ---- prologue: x tile 0 staging interleaved with W prep ----
    x0_pieces = [load_x_piece(0, pc) for pc in range(NPIECE)]
    w_pieces = {}
    for pc in range(NPIECE):
        for eb in range(2):
            wp = win_pool.tile([P, PIECE * P], F32, tag="win")
            nc.scalar.dma_start(
                wp, w_d[eb * P:(eb + 1) * P, pc * PIECE * P:(pc + 1) * PIECE * P])
            w_pieces[(eb, pc)] = wp

    xh_cur = xh_pool.tile([P, HC, P], FP16, tag="xh")
    xl8_cur = xh_pool.tile([P, HC, P], FP8, tag="xl8")
    x8_cur = xh_pool.tile([P, HC, P], FP8H, tag="x8")
    for u in range(NU):
        emit_x_unit(x0_pieces, u, xh_cur, xl8_cur, x8_cur)
        emit_w_unit(w_pieces[(0, u)], 0, u * UCH)
        emit_w_unit(w_pieces[(1, u)], 1, u * UCH)

    wout_sb = out_pool.tile([P, nt, TOPK], F32)
    iout_sb = out_pool.tile([P, nt, TOPK], U32)

    # ---- main loop over token tiles ----
    for i in range(nt):
        xh_p, xl8_p, x8_p = xh_cur, xl8_cur, x8_cur
        if i + 1 < nt:
            nxt_pieces = [load_x_piece(i + 1, pc) for pc in range(NPIECE)]
            xh_cur = xh_pool.tile([P, HC, P], FP16, tag="xh")
            xl8_cur = xh_pool.tile([P, HC, P], FP8, tag="xl8")
            x8_cur = xh_pool.tile([P, HC, P], FP8H, tag="x8")

        # phase B: fp16 main matmuls (uniform mode block)
        lg_m = ps_m_pool.tile([P, E], F32, tag="ps_m")
        for j in range(HC):
            nc.tensor.matmul(lg_m, xh_p[:, j, :], wh_sb[:, j, :],
                             start=(j == 0), stop=(j == HC - 1))

        # phase A for tile i+1 (fp32 transpose block)
        if i + 1 < nt:
            for u in range(NU):
                emit_x_unit(nxt_pieces, u, xh_cur, xl8_cur, x8_cur)

        # phase C: fp8e5 DoubleRow corrections (uniform mode block)
        lg_c = ps_c_pool.tile([P, E], F32, tag="ps_c")
        for pp in range(NPAIR):
            jsl = slice(2 * pp, 2 * pp + 2)
            nc.tensor.matmul(lg_c, xl8_p[:, jsl, :], w8_sb[:, jsl, :],
                             start=(pp == 0), stop=False,
                             perf_mode=mybir.MatmulPerfMode.DoubleRow)
            nc.tensor.matmul(lg_c, x8_p[:, jsl, :], wl8_sb[:, jsl, :],
                             start=False, stop=(pp == NPAIR - 1),
                             perf_mode=mybir.MatmulPerfMode.DoubleRow)

        # tail: logits = lg_m + lg_c/64; scores = sigmoid + bias
        corr_sc = sc_pool.tile([P, E], F32, tag="corr_sc")
        nc.scalar.mul(corr_sc, lg_c, S_INV)
        logits = sc_pool.tile([P, E], F32, tag="logits")
        nc.vector.tensor_add(logits, lg_m, corr_sc)
        scores = sc_pool.tile([P, E], F32, tag="scores")
        nc.scalar.activation(scores, logits, mybir.ActivationFunctionType.Sigmoid)
        nc.gpsimd.tensor_add(scores, scores, bias_rep)

        scores_g = scores.rearrange("p (g e) -> p g e", g=G)
        gmax = sm_pool.tile([P, G], F32, tag="gmax")
        nc.vector.reduce_max(gmax, scores_g, axis=mybir.AxisListType.X)

        g8 = sm_pool.tile([P, 8], F32, tag="g8")
        nc.vector.max(out=g8, in_=gmax)

        gmask = sm_pool.tile([P, G], F32, tag="gmask")
        nc.vector.tensor_scalar(gmask, gmax, g8[:, TOPK_G - 1:TOPK_G], None,
                                op0=mybir.AluOpType.is_ge)

        masked = sc_pool.tile([P, E], F32, tag="masked")
        nc.gpsimd.tensor_tensor(
            masked.rearrange("p (g e) -> p g e", g=G), scores_g,
            gmask[:, :, None].to_broadcast([P, G, GSZ]),
            op=mybir.AluOpType.mult)

        m8 = sm_pool.tile([P, 8], F32, tag="m8")
        nc.vector.max(out=m8, in_=masked)
        nc.vector.max_index(iout_sb[:, i, :], m8, masked)

        ssum = sm_pool.tile([P, 1], F32, tag="ssum")
        nc.vector.reduce_sum(ssum, m8, axis=mybir.AxisListType.X)
        nc.vector.tensor_scalar_add(ssum, ssum, 1e-6)
        rcp = sm_pool.tile([P, 1], F32, tag="rcp")
        nc.vector.reciprocal(rcp, ssum)
        nc.vector.tensor_scalar_mul(wout_sb[:, i, :], m8, rcp)

    nc.sync.dma_start(wout_d, wout_sb)
    nc.sync.dma_start(iout_d, iout_sb)


def build_bass(t_core=T_CORE):
    from concourse import bacc
    nc = bacc.Bacc("TRN2", target_bir_lowering=False, debug=False,
                   num_devices=N_CORES)
    nt = t_core // P
    x_d = nc.dram_tensor("x", [t_core, H], F32, kind="ExternalInput").ap()
    w_d = nc.dram_tensor("w", [E, H], F32, kind="ExternalInput").ap()
    b_d = nc.dram_tensor("b", [E], F32, kind="ExternalInput").ap()
    wout_d = nc.dram_tensor("wout", [P, nt, TOPK], F32,
                            kind="ExternalOutput").ap()
    iout_d = nc.dram_tensor("iout", [P, nt, TOPK], U32,
                            kind="ExternalOutput").ap()
    from contextlib import ExitStack
    with tile.TileContext(nc) as tc:
        with ExitStack() as ctx:
            build_moe_gate(tc, x_d, w_d, b_d, wout_d, iout_d, t_core, ctx=ctx)
    nc.compile()
    return nc


_NC_CACHE = {}


def _get_nc():
    key = "main"
    if key not in _NC_CACHE:
        _NC_CACHE[key] = build_bass()
    return _NC_CACHE[key]


def kernel(hidden_states, gate_weight, bias, n_group, topk_group, top_k,
           _trace=False):
    assert int(n_group) == G and int(topk_group) == TOPK_G and int(top_k) == TOPK
    x = np.asarray(hidden_states, dtype=np.float32)
    w = np.ascontiguousarray(np.asarray(gate_weight, dtype=np.float32))
    b = np.ascontiguousarray(np.asarray(bias, dtype=np.float32))
    B, S, _ = x.shape
    xf = x.reshape(-1, H)
    assert xf.shape[0] == T_FULL

    nc = _get_nc()
    in_maps = []
    for c in range(N_CORES):
        in_maps.append({
            "x": np.ascontiguousarray(xf[c * T_CORE:(c + 1) * T_CORE]),
            "w": w,
            "b": b,
        })
    try:
        res = run_bass_kernel_spmd(nc, in_maps, core_ids=list(range(N_CORES)),
                                   trace=_trace)
    except ModuleNotFoundError:
        res = run_bass_kernel_spmd(nc, in_maps, core_ids=list(range(N_CORES)),
                                   trace=False)
    weights = np.empty((T_FULL, TOPK), dtype=np.float32)
    indices = np.empty((T_FULL, TOPK), dtype=np.int32)
    for c, r in enumerate(res.results):
        wc = np.transpose(r["wout"], (1, 0, 2)).reshape(T_CORE, TOPK)
        ic = np.transpose(r["iout"], (1, 0, 2)).reshape(T_CORE, TOPK)
        weights[c * T_CORE:(c + 1) * T_CORE] = wc
        indices[c * T_CORE:(c + 1) * T_CORE] = ic.astype(np.int32)
    out_w = weights.reshape(B, S, TOPK)
    out_i = indices.reshape(B, S, TOPK)
    if _trace:
        return (out_w, out_i), res
    return out_w, out_i
